# revision 26
# baseline (speedup 1.0000x reference)
"""AngleAwareTripletLoss distributed Bass kernel v3 for 8 TRN2 NeuronCores.

Redesign vs v2 (167us):
  - All input DMAs dispatched from the Pool queue (25ns dispatch vs
    ~600ns DIRECT2D on sync) and big tables packed into few DMAs;
    window mining for all 4 blocks runs in the DMA shadow.
  - Similarity table shipped fp8-e4m3 (2MB) and the sim matmuls run
    fp8 DoubleRow (0.5 cyc/col) when USE_DR.
  - T = (900 - adist^2 - LAM*same) computed directly by the aug matmul
    (host-negated coefficients + host one-hot strip tables), so mining
    per 1024-col pair is: ACT Sign(T_psum) -> sgn fp16; one DVE
    tensor_tensor_reduce msim=(dot_psum + sgn) with fused max accum;
    one per-pair FIND_INDEX8 (pair-local argmax) that pipelines with
    the next pair, replacing the 4096-wide block scan.
  - Winning pair selected by a tiny [128,8] find over pair maxima;
    neg aux (nrm, angles) gathered from a packed fp16 row table.
  - Recon term via ACT Copy with accum (sum) over a host premultiplied
    fhat*fohat table - zero DVE cost.
  - Deferred triplet math batched into [128,4] ops after block 3.
"""

import sys
from contextlib import ExitStack

for _p in ("/opt/trn_rl_repo",):
    if _p not in sys.path:
        sys.path.insert(0, _p)

import numpy as np

import concourse.bass as bass
import concourse.mybir as mybir
from concourse.bass_utils import run_bass_kernel_spmd

B = 4096
F = 512
NCORES = 8
S = B // NCORES
P = 128
NB = S // P          # 4 blocks of 128 rows per core
NPAIR = 4            # 4 pairs of 1024 columns per block
LAM = 32768.0
WOFF = 192           # window start offset: block window = [bs-192, bs+320)
AUXW = 520           # packed row: [fhat 512 | nrm | a0 a1 a2 | pad4]
USE_DR = False       # fp8 DoubleRow was ~3x slower on HW; plain fp8

FP32 = mybir.dt.float32
FP16 = mybir.dt.float16
FP8 = mybir.dt.float8e4
FP8E5 = mybir.dt.float8e5
U32 = mybir.dt.uint32
AF = mybir.ActivationFunctionType
OP = mybir.AluOpType
AX = mybir.AxisListType
DR = mybir.MatmulPerfMode.DoubleRowSwInterleave

NP8 = mybir.dt.np(FP8)
NP8E5 = mybir.dt.np(FP8E5)

SEM_ENGINE = {
    "dBL": "gpsimd", "dGT": "sync", "dGT2": "sync",
    "dP0": "gpsimd", "dP1": "gpsimd",
    "dOut": "sync",
    "cPE": "tensor", "cACT": "scalar", "cDVE": "vector", "cPOOL": "gpsimd",
}
ASYNC_SEMS = {"dBL", "dGT", "dGT2", "dP0", "dP1", "dOut"}

# same-label strip chunks (512-col, rotated space) per block
STRIP = {0: [0, 1], 1: [1], 2: [1], 3: [1, 2]}
STRIP_LIST = [(m, ch) for m in range(NB) for ch in STRIP[m]]
STRIP_IDX = {mc: i for i, mc in enumerate(STRIP_LIST)}


class Sched:
    """Single-wait-per-instruction scheduler with buffer dep tracking."""

    def __init__(self, nc, stack):
        self.nc = nc
        self.sems = {k: stack.enter_context(nc.semaphore(name=f'sem_{k}'))
                     for k in SEM_ENGINE}
        self.counts = {k: 0 for k in SEM_ENGINE}
        self.hw = {}
        self.bufw = {}
        self.bufr = {}

    def _needed(self, eng, deps):
        best = {}
        for d in deps:
            if d is None:
                continue
            s, c = d
            if c <= 0:
                continue
            if s in ASYNC_SEMS:
                c = self.counts[s]
            if self.hw.get((eng, s), 0) >= c:
                continue
            best[s] = max(best.get(s, 0), c)
        return list(best.items())

    def run(self, sem, emit, *, n=1, reads=(), writes=(), extra=(),
            fuse=True):
        eng = SEM_ENGINE[sem]
        deps = []
        for b in reads:
            deps.append(self.bufw.get(b))
        for b in writes:
            deps.extend(self.bufr.get(b, []))
            deps.append(self.bufw.get(b))
        deps.extend(extra)
        gates = self._needed(eng, deps)
        if not fuse:
            for s, c in gates:
                getattr(self.nc, eng).wait_ge(self.sems[s], c)
                self.hw[(eng, s)] = c
            gates = []
        for s, c in gates[:-1]:
            getattr(self.nc, eng).wait_ge(self.sems[s], c)
            self.hw[(eng, s)] = c
        inst = emit()
        if gates:
            s, c = gates[-1]
            inst._wait_ge(self.sems[s], c)
            self.hw[(eng, s)] = c
        self.counts[sem] += n
        inst.then_inc(self.sems[sem], n)
        cur = (sem, self.counts[sem])
        for b in writes:
            self.bufw[b] = cur
            self.bufr[b] = []
        for b in reads:
            self.bufr.setdefault(b, []).append(cur)
        return cur


def build_graph():
    nc = bass.Bass(trn_type="TRN2", num_devices=NCORES)

    dp_ = nc.declare_dram_parameter
    # big tables
    gt8 = [dp_(f"gt8_{c}", [P, 4 * 1024], FP8, isOutput=False)
           for c in range(4)]                                  # [p][kt][1024]
    gtown8 = dp_("gtown8", [P, 4 * 128 * NB], FP8, isOutput=False)
    gtownI = dp_("gtownI", [P, 2 * NB * 256], FP8, isOutput=False)
    paux = dp_("paux", [B, AUXW], FP16, isOutput=False)        # pos gather
    paux2 = dp_("paux2", [B, 8], FP16, isOutput=False)         # neg gather
    aug = dp_("aug", [5, S + B + NB * 512], FP16, isOutput=False)
    blob16a = dp_("blob16a", [P, (NB + len(STRIP_LIST)) * 512 + P],
                  FP8E5, isOutput=False)  # winOH | stripOH | lamEye
    blob16b = dp_("blob16b", [P, 2 * NB * 512], FP16, isOutput=False)
    # fhA | prodRT
    blob32 = dp_("blob32", [P, 32], FP32, isOutput=False)
    # [wsb 4 | negoff 1 | pad3 | nrmA 4 | ssqA 4 | rang 12 | pad4]
    out = dp_("out", [1, 16], FP32, isOutput=True)

    sb_ = nc.alloc_sbuf_tensor
    GT8 = sb_("s_GT8", [P, 4 * 4 * 1024], FP8).ap()
    GT8v = GT8.rearrange("p (c k b) -> p c k b", c=4, k=4)
    gownS = sb_("s_gown", [P, 4 * 128 * NB], FP8).ap()
    gownV = gownS.rearrange("p (k s) -> p k s", k=4)
    gownIS = sb_("s_gownI", [P, 2 * NB * 256], FP8).ap()
    # [q 2][block 4][interleaved flat 256] - contiguous reshape view
    gownIV = gownIS.rearrange("p (q m x y) -> p q m x y", q=2, m=NB, x=2)
    augS = sb_("s_aug", [5, S + B + NB * 512], FP16).ap()
    augR = augS[:, 0:S]                          # [a;1;|a|^2] own rows
    augCT = augS[:, S:S + B]                     # [2a; 900-|a|^2; -1] rot cols
    augWw = augS[:, S + B:]                      # window D1 aug [5, 2048]
    b16a = sb_("s_b16a", [P, (NB + len(STRIP_LIST)) * 512 + P], FP8E5).ap()
    winOH = b16a[:, 0:NB * 512]
    stripOH = b16a[:, NB * 512:(NB + len(STRIP_LIST)) * 512]
    lamEye = b16a[:, (NB + len(STRIP_LIST)) * 512:]
    b16b = sb_("s_b16b", [P, 2 * NB * 512], FP16).ap()
    fhA = b16b[:, 0:NB * 512]
    prodRT = b16b[:, NB * 512:]
    b32 = sb_("s_b32", [P, 32], FP32).ap()
    wsb = b32[:, 0:4]
    negoff = b32[:, 4:5]
    nrmA = b32[:, 8:12]
    ssqA = b32[:, 12:16]
    rang = b32[:, 16:28]

    msim = [sb_(f"s_msim{i}", [P, B], FP16).ap() for i in range(2)]
    sgnb = [sb_(f"s_sgn{i}", [P, 1024], FP16).ap() for i in range(2)]
    dotc = [sb_(f"s_dotc{i}", [P, 1024], FP16).ap() for i in range(2)]
    winsb = sb_("s_winsb", [P, NB * 512], FP32).ap()
    mch8x = [sb_(f"s_mch8x{i}", [P, NPAIR * 8], FP16).ap() for i in range(2)]
    mchd = sb_("s_mchd", [P, 8], FP16).ap()
    M18h = sb_("s_M18h", [P, 8], FP16).ap()
    idx8 = [sb_(f"s_idx8{i}", [P, NPAIR * 8], U32).ap() for i in range(2)]
    M8w = sb_("s_M8w", [P, 8], FP32).ap()
    idxw8 = sb_("s_idxw8", [P, NB * 8], U32).ap()
    Mw = sb_("s_Mw", [P, NB], FP32).ap()
    haspw = sb_("s_haspw", [P, NB], FP32).ap()
    posf = sb_("s_posf", [P, NB], FP32).ap()
    posu = sb_("s_posu", [P, NB], U32).ap()
    phP = sb_("s_phP", [P, NB * AUXW], FP16).ap()
    aux8 = sb_("s_aux8", [P, NB * 8], FP16).ap()
    M1 = sb_("s_M1", [P, 1], FP32).ap()
    prsel8 = sb_("s_prsel8", [P, 8], U32).ap()
    prself = sb_("s_prself", [P, 1], FP32).ap()
    prconst = sb_("s_prconst", [P, 4], FP32).ap()
    seleq = sb_("s_seleq", [P, 4], FP32).ap()
    idxlf = sb_("s_idxlf", [P, 4], FP32).ap()
    selidx = sb_("s_selidx", [P, 4], FP32).ap()
    negl = sb_("s_negl", [P, 1], FP32).ap()
    negf = sb_("s_negf", [P, 1], FP32).ap()
    geT = sb_("s_geT", [P, 1], FP32).ap()
    negu = sb_("s_negu", [P, NB], U32).ap()
    dotp = sb_("s_dotp", [P, NB], FP32).ap()
    dotn = sb_("s_dotn", [P, NB], FP32).ap()
    z8f = sb_("s_z8f", [P, 8], FP32).ap()
    racc = sb_("s_racc", [P, 1], FP32).ap()
    scrD = sb_("s_scrD", [P, 1024], FP16).ap()     # ttr junk out
    d3 = sb_("s_d3", [P, 12], FP32).ap()
    d3sq = sb_("s_d3sq", [P, 12], FP32).ap()
    d3n = sb_("s_d3n", [P, 12], FP32).ap()
    d3nsq = sb_("s_d3nsq", [P, 12], FP32).ap()
    t3n = sb_("s_t3n", [P, NB], FP32).ap()
    pasq = sb_("s_pasq", [P, NB], FP32).ap()
    nasq = sb_("s_nasq", [P, NB], FP32).ap()
    nrmP = sb_("s_nrmP", [P, NB], FP32).ap()
    nrmN = sb_("s_nrmN", [P, NB], FP32).ap()
    t1 = sb_("s_t1", [P, NB], FP32).ap()
    t2 = sb_("s_t2", [P, NB], FP32).ap()
    t3 = sb_("s_t3", [P, NB], FP32).ap()
    pnd = sb_("s_pnd", [P, 2 * NB], FP32).ap()     # [pdq | ndq]
    w1q = sb_("s_w1q", [P, NB], FP32).ap()
    w2q = sb_("s_w2q", [P, NB], FP32).ap()
    bq = sb_("s_bq", [P, NB], FP32).ap()
    wbq = sb_("s_wbq", [P, NB], FP32).ap()
    a_s1 = sb_("s_a_s1", [P, 4], FP32).ap()
    onesP = sb_("s_onesP", [P, 1], FP32).ap()
    part_sb = sb_("s_part", [1, 16], FP32).ap()

    pT = [nc.alloc_psum_tensor(f"p_T{i}", [P, 1024], FP32).ap()
          for i in range(2)]
    pD = [nc.alloc_psum_tensor(f"p_D{i}", [P, 1024], FP32).ap()
          for i in range(2)]

    with ExitStack() as stack:
        sc = Sched(nc, stack)
        sy, ve, ac, te, gp = nc.sync, nc.vector, nc.scalar, nc.tensor, nc.gpsimd

        def dma(dst, src, buf, sem="dBL", reads=()):
            eng = getattr(nc, SEM_ENGINE[sem])
            return sc.run(sem, lambda: eng.dma_start(dst, src), n=16,
                          writes=(buf,), reads=reads)

        # ---------------- memsets ----------------
        sc.run("cDVE", lambda: ve.memset(z8f[:], 0.0), writes=("z8f",))
        sc.run("cDVE", lambda: ve.memset(mchd[:], -30000.0), writes=("mchd_pad",))
        sc.run("cDVE", lambda: ve.memset(onesP[:], 1.0), writes=("onesP",))
        sc.run("cDVE", lambda: ve.memset(part_sb[:], 0.0), writes=("part_sb",))
        sc.run("cDVE", lambda: ve.memset(prconst[:, 0:1], 0.0), writes=("pc0",))
        sc.run("cDVE", lambda: ve.memset(prconst[:, 1:2], 1.0), writes=("pc1",))
        sc.run("cDVE", lambda: ve.memset(prconst[:, 2:3], 2.0), writes=("pc2",))
        sc.run("cDVE", lambda: ve.memset(prconst[:, 3:4], 3.0), writes=("pc3",))

        # ------- input DMAs: window tables first (Pool queue) -------
        dma(b32[:], blob32[:, :], "b32")
        dma(augS[:], aug[:, :], "aug")
        dma(b16a[:], blob16a[:, :], "b16a")

        # ---------------- windows (all 4 blocks, DMA shadow) ----------------
        for m in range(NB):
            pW = pT[m % 2][:, 0:512]
            pwb = f"pT{m % 2}"
            sc.run("cPE", lambda m=m, pW=pW: te.matmul(
                pW, augR[:, m * P:(m + 1) * P],
                augWw[:, m * 512:(m + 1) * 512], start=True, stop=False),
                reads=("aug",), writes=(pwb,))
            sc.run("cPE", lambda m=m, pW=pW: te.matmul(
                pW, lamEye[:], winOH[:, m * 512:(m + 1) * 512],
                start=False, stop=True),
                reads=("b16a",), writes=(pwb,))
            sc.run("cACT", lambda m=m, pW=pW: ac.activation(
                winsb[:, m * 512:(m + 1) * 512], pW, AF.Copy),
                reads=(pwb,), writes=(f"win{m}",))
            sc.run("cDVE", lambda m=m: ve.tensor_reduce(
                Mw[:, m:m + 1], winsb[:, m * 512:(m + 1) * 512],
                axis=AX.X, op=OP.max),
                reads=(f"win{m}",), writes=(f"Mw{m}",))
            sc.run("cDVE", lambda m=m: ve.tensor_scalar(
                M8w[:], z8f[:], Mw[:, m:m + 1], None, op0=OP.add),
                reads=(f"Mw{m}", "z8f"), writes=("M8w",))
            sc.run("cDVE", lambda m=m: ve.max_index(
                idxw8[:, m * 8:(m + 1) * 8], M8w[:],
                winsb[:, m * 512:(m + 1) * 512]),
                reads=("M8w", f"win{m}"), writes=(f"idxw{m}",))
        # batched pos chain
        sc.run("cDVE", lambda: ve.tensor_scalar(
            haspw[:], Mw[:], 28000.0, None, op0=OP.is_gt),
            reads=("Mw0", "Mw1", "Mw2", "Mw3"), writes=("haspw",))
        idxw_s = idxw8.rearrange("p (m e) -> p m e", m=NB)[:, :, 0]
        sc.run("cDVE", lambda: ve.tensor_copy(posf[:], idxw_s),
               reads=("idxw0", "idxw1", "idxw2", "idxw3"), writes=("posf",))
        sc.run("cDVE", lambda: ve.tensor_tensor(
            posf[:], posf[:], wsb[:], op=OP.add),
            reads=("posf", "b32"), writes=("posf",))
        sc.run("cDVE", lambda: ve.tensor_scalar(
            posf[:], posf[:], 0.0, float(B - 1), op0=OP.max, op1=OP.min),
            reads=("posf",), writes=("posf",))
        sc.run("cDVE", lambda: ve.tensor_copy(posu[:], posf[:]),
               reads=("posf",), writes=("posu",))
        for m in range(NB):
            sc.run("dP0", lambda m=m: gp.indirect_dma_start(
                phP[:, m * AUXW:(m + 1) * AUXW], None, paux[:, :],
                bass.IndirectOffsetOnAxis(ap=posu[:, m:m + 1], axis=0)),
                n=16, reads=("posu",), writes=(f"phP{m}",))

        # ------- big tables (sync queue; overlap with windows) -------
        # ensure the blob DMAs enter the HW queues first
        nc.sync.wait_ge(sc.sems["dBL"], sc.counts["dBL"])
        sc.hw[("sync", "dBL")] = sc.counts["dBL"]
        dma(gownS[:], gtown8[:, :], "gown", sem="dGT")
        dma(gownIS[:], gtownI[:, :], "gownI", sem="dGT")
        for c in range(4):
            dma(GT8[:, c * 4096:(c + 1) * 4096], gt8[c][:, :], f"GT{c}",
                sem="dGT")
        dma(b16b[:], blob16b[:, :], "b16b", sem="dGT2")

        # ---------------- recon (ACT accum over premultiplied table) -------
        sc.run("cACT", lambda: ac.activation(
            msim[0][:, 0:NB * 512], prodRT[:], AF.Copy, accum_out=racc[:]),
            reads=("b16b",), writes=("racc", "ms0_0", "ms0_1"))

        # ------- pos dots + pos-side chain (mining shadow) -------
        AUX = tuple(f"aux8_{m}" for m in range(NB))
        PHP = tuple(f"phP{m}" for m in range(NB))
        phPv = phP.rearrange("p (m w) -> p m w", m=NB)
        aux8v = aux8.rearrange("p (m w) -> p m w", m=NB)
        aP = phPv[:, :, 513:516]
        aN = aux8v[:, :, 1:4]
        nrmPs = phPv[:, :, 512]
        nrmNs = aux8v[:, :, 0]
        rangv = rang.rearrange("p (m w) -> p m w", w=3)
        d3v3 = d3.rearrange("p (m w) -> p m w", w=3)
        d3sqv = d3sq.rearrange("p (m w) -> p m w", w=3)
        d3nv3 = d3n.rearrange("p (m w) -> p m w", w=3)
        d3nsqv = d3nsq.rearrange("p (m w) -> p m w", w=3)
        for m in range(NB):
            sc.run("cDVE", lambda m=m: ve.scalar_tensor_tensor(
                scrD[:, 0:512], phP[:, m * AUXW:m * AUXW + 512], 1.0,
                fhA[:, m * 512:(m + 1) * 512],
                op0=OP.mult, op1=OP.mult, accum_out=dotp[:, m:m + 1]),
                reads=(f"phP{m}", "b16b"), writes=("dotp", "scrD"))
        sc.run("cDVE", lambda: ve.tensor_tensor(
            d3v3, rangv, aP, op=OP.subtract),
            reads=("b32",) + PHP, writes=("d3",))
        sc.run("cDVE", lambda: ve.tensor_tensor(
            d3sq[:], d3[:], d3[:], op=OP.mult),
            reads=("d3",), writes=("d3sq",))
        sc.run("cDVE", lambda: ve.tensor_reduce(
            pasq[:], d3sqv, axis=AX.X, op=OP.add),
            reads=("d3sq",), writes=("pasq",))
        sc.run("cDVE", lambda: ve.tensor_copy(nrmP[:], nrmPs),
               reads=PHP, writes=("nrmP",))
        sc.run("cDVE", lambda: ve.tensor_tensor(
            t1[:], nrmA[:], nrmP[:], op=OP.mult),
            reads=("b32", "nrmP"), writes=("t1",))
        sc.run("cDVE", lambda: ve.tensor_tensor(
            t2[:], t1[:], dotp[:], op=OP.mult),
            reads=("t1", "dotp"), writes=("t2",))
        sc.run("cDVE", lambda: ve.tensor_tensor(
            t3[:], nrmP[:], nrmP[:], op=OP.mult),
            reads=("nrmP",), writes=("t3",))
        sc.run("cDVE", lambda: ve.tensor_tensor(
            t3[:], ssqA[:], t3[:], op=OP.add),
            reads=("t3", "b32"), writes=("t3",))
        sc.run("cDVE", lambda: ve.scalar_tensor_tensor(
            pnd[:, 0:4], t2[:], -2.0, t3[:], op0=OP.mult, op1=OP.add),
            reads=("t2", "t3"), writes=("pndp",))
        sc.run("cDVE", lambda: ve.tensor_scalar(
            w1q[:], pasq[:], 2025.0, 1.0, op0=OP.is_gt, op1=OP.add),
            reads=("pasq",), writes=("w1q",))

        # ---------------- mining ----------------
        def emit_pair(m, pr):
            par = m % 2
            pb = pr % 2
            pTb, pDb = pT[pb], pD[pb]
            tn, dn_ = f"pT{pb}", f"pD{pb}"
            sg = sgnb[pb]
            for half in range(2):
                ch = pr * 2 + half
                cc, off = ch // 2, (ch % 2) * 512
                strips = (m, ch) in STRIP_IDX

                def mm_T(m=m, half=half, ch=ch, pTb=pTb, strips=strips):
                    return te.matmul(
                        pTb[:, half * 512:(half + 1) * 512],
                        augR[:, m * P:(m + 1) * P],
                        augCT[:, ch * 512:(ch + 1) * 512],
                        start=True, stop=not strips)
                sc.run("cPE", mm_T, reads=("aug",), writes=(tn,))
                if strips:
                    si = STRIP_IDX[(m, ch)]

                    def mm_s(si=si, half=half, pTb=pTb):
                        return te.matmul(
                            pTb[:, half * 512:(half + 1) * 512], lamEye[:],
                            stripOH[:, si * 512:(si + 1) * 512],
                            start=False, stop=True)
                    sc.run("cPE", mm_s, reads=("b16a",), writes=(tn,))

                if USE_DR:
                    for q in range(2):
                        def mm_d(m=m, q=q, cc=cc, off=off, pDb=pDb,
                                 half=half):
                            return te.matmul(
                                pDb[:, half * 512:(half + 1) * 512],
                                gownIV[:, q, m],
                                GT8v[:, cc, 2 * q:2 * q + 2,
                                     off:off + 512],
                                start=(q == 0), stop=(q == 1),
                                perf_mode=DR)
                        sc.run("cPE", mm_d, reads=("gownI", f"GT{cc}"),
                               writes=(dn_,))
                else:
                    for q in range(4):
                        def mm_d(m=m, q=q, cc=cc, off=off, pDb=pDb,
                                 half=half):
                            return te.matmul(
                                pDb[:, half * 512:(half + 1) * 512],
                                gownV[:, q, m * P:(m + 1) * P],
                                GT8v[:, cc, q, off:off + 512],
                                start=(q == 0), stop=(q == 3))
                        sc.run("cPE", mm_d, reads=("gown", f"GT{cc}"),
                               writes=(dn_,))

            sc.run("cACT", lambda pTb=pTb, sg=sg: ac.activation(
                sg[:], pTb[:], AF.Sign),
                reads=(tn,), writes=(f"sgn{pb}",))
            sc.run("cACT", lambda pDb=pDb, pb=pb: ac.activation(
                dotc[pb][:], pDb[:], AF.Copy),
                reads=(dn_,), writes=(f"dotc{pb}",))
            sc.run("cPOOL", lambda par=par, pr=pr, pb=pb, sg=sg:
                   gp.tensor_tensor(
                       msim[par][:, pr * 1024:(pr + 1) * 1024],
                       dotc[pb][:], sg[:], op=OP.add),
                   reads=(f"dotc{pb}", f"sgn{pb}"),
                   writes=(f"ms{par}_{pr}",))
            sc.run("cDVE", lambda par=par, pr=pr: ve.max(
                mch8x[par][:, pr * 8:(pr + 1) * 8],
                msim[par][:, pr * 1024:(pr + 1) * 1024]),
                reads=(f"ms{par}_{pr}",), writes=(f"mch{par}_{pr}",))
            sc.run("cDVE", lambda par=par, pr=pr: ve.max_index(
                idx8[par][:, pr * 8:(pr + 1) * 8],
                mch8x[par][:, pr * 8:(pr + 1) * 8],
                msim[par][:, pr * 1024:(pr + 1) * 1024]),
                reads=(f"mch{par}_{pr}", f"ms{par}_{pr}"),
                writes=(f"idx{par}_{pr}",))

        def emit_block_tail(m):
            par = m % 2
            MCH = tuple(f"mch{par}_{p}" for p in range(NPAIR))
            IDX = tuple(f"idx{par}_{p}" for p in range(NPAIR))
            mch_s = mch8x[par].rearrange("p (q e) -> p q e", q=NPAIR)[:, :, 0]
            sc.run("cDVE", lambda mch_s=mch_s: ve.tensor_copy(
                mchd[:, 0:4], mch_s), reads=MCH + ("mchd_pad",),
                writes=("mchd",))
            sc.run("cDVE", lambda: ve.tensor_reduce(
                M1[:], mchd[:, 0:4], axis=AX.X, op=OP.max),
                reads=("mchd",), writes=("M1",))
            sc.run("cDVE", lambda: ve.tensor_scalar(
                M18h[:], z8f[:], M1[:, 0:1], None, op0=OP.add),
                reads=("M1", "z8f"), writes=("M18h",))
            sc.run("cDVE", lambda: ve.max_index(
                prsel8[:], M18h[:], mchd[:]),
                reads=("M18h", "mchd", "mchd_pad"), writes=("prsel8",))
            sc.run("cDVE", lambda m=m: ve.tensor_scalar(
                dotn[:, m:m + 1], M1[:], -1.0, None, op0=OP.add),
                reads=("M1",), writes=("dotn",))
            sc.run("cPOOL", lambda: gp.tensor_copy(prself[:], prsel8[:, 0:1]),
                   reads=("prsel8",), writes=("prself",))
            idx_s = idx8[par].rearrange("p (q e) -> p q e", q=NPAIR)[:, :, 0]
            sc.run("cDVE", lambda idx_s=idx_s: ve.tensor_copy(
                idxlf[:], idx_s), reads=IDX, writes=("idxlf",))
            sc.run("cPOOL", lambda: gp.tensor_scalar(
                seleq[:], prconst[:], prself[:, 0:1], None, op0=OP.is_equal),
                reads=("prself", "pc0", "pc1", "pc2", "pc3"),
                writes=("seleq",))
            sc.run("cPOOL", lambda: gp.tensor_tensor(
                selidx[:], seleq[:], idxlf[:], op=OP.mult),
                reads=("seleq", "idxlf"), writes=("selidx",))
            sc.run("cDVE", lambda: ve.tensor_reduce(
                negl[:], selidx[:], axis=AX.X, op=OP.add),
                reads=("selidx",), writes=("negl",))
            sc.run("cDVE", lambda: ve.scalar_tensor_tensor(
                negf[:], prself[:], 1024.0, negl[:],
                op0=OP.mult, op1=OP.add),
                reads=("prself", "negl"), writes=("negf",))
            sc.run("cDVE", lambda: ve.tensor_scalar(
                negf[:], negf[:], negoff[:, 0:1], None, op0=OP.add),
                reads=("negf", "b32"), writes=("negf",))
            sc.run("cDVE", lambda: ve.tensor_scalar(
                geT[:], negf[:], float(B), None, op0=OP.is_ge),
                reads=("negf",), writes=("geT",))
            sc.run("cDVE", lambda: ve.scalar_tensor_tensor(
                negf[:], geT[:], -float(B), negf[:],
                op0=OP.mult, op1=OP.add),
                reads=("geT", "negf"), writes=("negf",))
            sc.run("cDVE", lambda m=m: ve.tensor_copy(
                negu[:, m:m + 1], negf[:]),
                reads=("negf",), writes=("negu",))
            sc.run("dP1", lambda m=m: gp.indirect_dma_start(
                aux8[:, m * 8:(m + 1) * 8], None, paux2[:, :],
                bass.IndirectOffsetOnAxis(ap=negu[:, m:m + 1], axis=0)),
                n=16, reads=("negu",), writes=(f"aux8_{m}",))

        for m in range(NB):
            for pr in range(NPAIR):
                emit_pair(m, pr)
            emit_block_tail(m)

        # ---------------- neg-side tail (split Pool/DVE) ----------------
        sc.run("cPOOL", lambda: gp.tensor_tensor(
            d3nv3, rangv, aN, op=OP.subtract),
            reads=("b32",) + AUX, writes=("d3n",))
        sc.run("cPOOL", lambda: gp.tensor_tensor(
            d3nsq[:], d3n[:], d3n[:], op=OP.mult),
            reads=("d3n",), writes=("d3nsq",))
        sc.run("cPOOL", lambda: gp.tensor_copy(nrmN[:], nrmNs),
               reads=AUX, writes=("nrmN",))
        sc.run("cPOOL", lambda: gp.tensor_tensor(
            t3n[:], nrmN[:], nrmN[:], op=OP.mult),
            reads=("nrmN",), writes=("t3n",))
        sc.run("cPOOL", lambda: gp.tensor_tensor(
            t3n[:], ssqA[:], t3n[:], op=OP.add),
            reads=("t3n", "b32"), writes=("t3n",))
        sc.run("cDVE", lambda: ve.tensor_reduce(
            nasq[:], d3nsqv, axis=AX.X, op=OP.add),
            reads=("d3nsq",), writes=("nasq",))
        sc.run("cPOOL", lambda: gp.tensor_scalar(
            w2q[:], nasq[:], 225.0, None, op0=OP.is_lt),
            reads=("nasq",), writes=("w2q",))
        sc.run("cPOOL", lambda: gp.tensor_scalar(
            w2q[:], w2q[:], 0.5, 1.0, op0=OP.mult, op1=OP.add),
            reads=("w2q",), writes=("w2q",))
        sc.run("cDVE", lambda: ve.tensor_tensor(
            t1[:], nrmA[:], nrmN[:], op=OP.mult),
            reads=("b32", "nrmN"), writes=("t1n",))
        sc.run("cDVE", lambda: ve.tensor_tensor(
            t2[:], t1[:], dotn[:], op=OP.mult),
            reads=("t1n", "dotn"), writes=("t2n",))
        sc.run("cDVE", lambda: ve.scalar_tensor_tensor(
            pnd[:, 4:8], t2[:], -2.0, t3n[:], op0=OP.mult, op1=OP.add),
            reads=("t2n", "t3n"), writes=("pndn",))
        sc.run("cDVE", lambda: ve.tensor_scalar_max(pnd[:], pnd[:], 0.0),
               reads=("pndp", "pndn"), writes=("pndp", "pndn"))
        sc.run("cACT", lambda: ac.activation(pnd[:], pnd[:], AF.Sqrt),
               reads=("pndp", "pndn"), writes=("pndp", "pndn"))
        sc.run("cPOOL", lambda: gp.tensor_tensor(
            w1q[:], w1q[:], w2q[:], op=OP.mult),
            reads=("w1q", "w2q"), writes=("w1q",))
        sc.run("cDVE", lambda: ve.tensor_sub(bq[:], pnd[:, 0:4], pnd[:, 4:8]),
               reads=("pndp", "pndn"), writes=("bq",))
        sc.run("cDVE", lambda: ve.tensor_scalar(
            bq[:], bq[:], 0.2, 0.0, op0=OP.add, op1=OP.max),
            reads=("bq",), writes=("bq",))
        sc.run("cDVE", lambda: ve.tensor_tensor(
            wbq[:], w1q[:], bq[:], op=OP.mult),
            reads=("w1q", "bq"), writes=("wbq",))
        sc.run("cDVE", lambda: ve.tensor_tensor(
            wbq[:], wbq[:], haspw[:], op=OP.mult),
            reads=("wbq", "haspw"), writes=("wbq",))
        sc.run("cDVE", lambda: ve.tensor_reduce(
            a_s1[:, 0:1], wbq[:], axis=AX.X, op=OP.add),
            reads=("wbq",), writes=("acc0",))
        sc.run("cDVE", lambda: ve.tensor_reduce(
            a_s1[:, 1:2], haspw[:], axis=AX.X, op=OP.add),
            reads=("haspw",), writes=("acc1",))
        sc.run("cDVE", lambda: ve.tensor_copy(a_s1[:, 2:3], racc[:]),
               reads=("racc",), writes=("acc2",))
        sc.run("cDVE", lambda: ve.memset(a_s1[:, 3:4], 0.0),
               writes=("acc3",))

        # ---------------- partition reduce + out ----------------
        sc.run("cPE", lambda: te.matmul(pD[0][:1, 0:4], onesP[:], a_s1[:],
                                        start=True, stop=True),
               reads=("onesP", "acc0", "acc1", "acc2", "acc3"),
               writes=("pD0",))
        sc.run("cACT", lambda: ac.activation(part_sb[:1, 0:4],
                                             pD[0][:1, 0:4], AF.Copy),
               reads=("pD0", "part_sb"), writes=("part_sb",))
        sc.run("dOut", lambda: sy.dma_start(out[:, :], part_sb[:]),
               n=16, reads=("part_sb",), writes=("out",))
        nc.sync.wait_ge(sc.sems["dOut"], sc.counts["dOut"])
        nc.all_engine_barrier()

    return nc


_cached = {}


def _prep(features, labels, angles, features_orig):
    f = np.ascontiguousarray(np.asarray(features, dtype=np.float32))
    ang = np.ascontiguousarray(np.asarray(angles, dtype=np.float32))
    fo = np.ascontiguousarray(np.asarray(features_orig, np.float32))
    lab = np.asarray(labels)

    perm = np.argsort(lab, kind="stable")
    f = f[perm]
    lab = lab[perm].astype(np.float32)
    ang = ang[perm]
    fo = fo[perm]

    assert np.max(np.bincount(np.asarray(labels).astype(np.int64))) <= P

    nrm = np.sqrt((f * f).sum(1))
    fhat = (f / nrm[:, None]).astype(np.float16)
    fonrm = np.sqrt((fo * fo).sum(1))
    fohat = (fo / fonrm[:, None]).astype(np.float16)
    ssq = (f.astype(np.float64) ** 2).sum(1).astype(np.float32)
    fhatT = fhat.T.astype(np.float32)             # [F, B]
    fhat8T = fhatT.astype(NP8)                    # [F, B] fp8
    lab16 = lab.astype(np.float16)
    a2 = (ang.astype(np.float16).astype(np.float32) ** 2).sum(1)

    # packed gather table [B, 520]: fhat | nrm | a0 a1 a2 | pad
    paux_np = np.zeros((B, AUXW), np.float16)
    paux_np[:, 0:512] = fhat
    paux_np[:, 512] = nrm.astype(np.float16)
    paux_np[:, 513:516] = ang.astype(np.float16)
    paux2_np = np.zeros((B, 8), np.float16)
    paux2_np[:, 0] = nrm.astype(np.float16)
    paux2_np[:, 1:4] = ang.astype(np.float16)

    in_maps = []
    for c in range(NCORES):
        r0 = c * S
        rot = (np.arange(B) + (c - 1) * 512) % B    # rel col -> global col
        labr = lab[rot]
        # gt8: [colchunk 4][p 128][kt 4][1024] contiguous per chunk
        g = fhat8T.reshape(4, P, B)[:, :, rot]      # [kt, p, col]
        gt8_c = {}
        for cc in range(4):
            blk = g[:, :, cc * 1024:(cc + 1) * 1024]      # [kt, p, 1024]
            gt8_c[cc] = np.ascontiguousarray(
                blk.transpose(1, 0, 2).reshape(P, 4 * 1024))
        gtown_c = np.ascontiguousarray(
            fhat8T.reshape(4, P, B)[:, :, r0:r0 + S]
            .transpose(1, 0, 2).reshape(P, 4 * S))
        # SwInterleave stationary: per (q, block): interleave(A[:, ::-1],
        # B[:, ::-1]) where A/B are the two kt tiles' [128, 128] col blocks
        gI = fhat8T.reshape(4, P, B)[:, :, r0:r0 + S]   # [kt, p, S]
        gtownI_c = np.zeros((P, 2, NB, 256), NP8)
        for q in range(2):
            for m in range(NB):
                A = gI[2 * q, :, m * P:(m + 1) * P]      # [p, 128]
                Bt = gI[2 * q + 1, :, m * P:(m + 1) * P]
                F = np.stack([A[:, ::-1], Bt[:, ::-1]], axis=-1)
                gtownI_c[:, q, m, :] = F.reshape(P, 256)
        gtownI_c = np.ascontiguousarray(gtownI_c.reshape(P, 2 * NB * 256))
        # aug tables (fp16): augR [a;1;|a|^2] own; augCT [2a; 900-|a|^2; -1]
        rang_c = ang[r0:r0 + S]
        augR_c = np.concatenate(
            [rang_c.T, np.ones((1, S), np.float32),
             (a2[r0:r0 + S])[None, :]], axis=0)
        augCT_c = np.concatenate(
            [2.0 * ang.T, (900.0 - a2)[None, :],
             -np.ones((1, B), np.float32)], axis=0)[:, rot]
        # window aug: D1 = augR . [ -2a; |a|^2; 1 ] over window cols
        augW_c = np.zeros((5, NB, 512), np.float32)
        winOH_c = np.zeros((P, NB, 512), np.float16)
        wsb_c = np.zeros(NB, np.float32)
        for m in range(NB):
            bs = r0 + m * P
            ws = bs - WOFF
            wsb_c[m] = float(ws)
            lo, hi = max(ws, 0), min(ws + 512, B)
            augW_c[0:3, m, lo - ws:hi - ws] = -2.0 * ang[lo:hi].T
            augW_c[3, m, lo - ws:hi - ws] = a2[lo:hi]
            augW_c[4, m, lo - ws:hi - ws] = 1.0
            # window one-hot: (2*self - eq) so that matmul with -LAM*I
            # gives +LAM*same - 2*LAM*self
            eq = (lab[bs:bs + P, None] == lab[None, lo:hi]).astype(np.float32)
            oh = np.zeros((P, 512), np.float32)
            oh[:, lo - ws:hi - ws] = -eq
            oh[np.arange(P), WOFF + np.arange(P)] += 2.0
            winOH_c[:, m, :] = oh.astype(np.float16)
        # strip one-hots (same-label mask per block/chunk, rotated cols)
        stripOH_c = np.zeros((P, len(STRIP_LIST), 512), np.float16)
        for i, (m, ch) in enumerate(STRIP_LIST):
            bs = r0 + m * P
            stripOH_c[:, i, :] = (
                lab[bs:bs + P, None] == labr[None, ch * 512:(ch + 1) * 512]
            ).astype(np.float16)
        # verify strips cover all same-label columns
        # (cheap insurance; rotation makes this static)
        lamEye_c = (-LAM * np.eye(P)).astype(np.float16)
        fhA_c = np.ascontiguousarray(fhat[r0:r0 + S].reshape(NB, P, 512)
                                     .transpose(1, 0, 2).reshape(P, NB * 512))
        prod = (fhat[r0:r0 + S].astype(np.float32) *
                fohat[r0:r0 + S].astype(np.float32))
        prodRT_c = np.ascontiguousarray(
            prod.reshape(NB, P, 512).transpose(1, 0, 2)
            .reshape(P, NB * 512)).astype(np.float16)
        blob32_c = np.zeros((P, 32), np.float32)
        blob32_c[:, 0:4] = wsb_c[None, :]
        blob32_c[:, 4] = float(((c - 1) % NCORES) * 512)
        blob32_c[:, 8:12] = nrm[r0:r0 + S].reshape(NB, P).T
        blob32_c[:, 12:16] = ssq[r0:r0 + S].reshape(NB, P).T
        blob32_c[:, 16:28] = rang_c.reshape(NB, P, 3).transpose(1, 0, 2)\
            .reshape(P, NB * 3)
        b16a_c = np.concatenate(
            [winOH_c.reshape(P, NB * 512).astype(np.float32),
             stripOH_c.reshape(P, len(STRIP_LIST) * 512).astype(np.float32),
             lamEye_c.astype(np.float32)], axis=1).astype(NP8E5)
        b16b_c = np.concatenate([fhA_c, prodRT_c], axis=1)
        aug_c = np.concatenate(
            [augR_c, augCT_c, augW_c.reshape(5, NB * 512)],
            axis=1).astype(np.float16)
        im = {
            "gtown8": gtown_c,
            "gtownI": gtownI_c,
            "paux": paux_np,
            "paux2": paux2_np,
            "aug": aug_c,
            "blob16a": np.ascontiguousarray(b16a_c),
            "blob16b": np.ascontiguousarray(b16b_c),
            "blob32": blob32_c,
        }
        for cc in range(4):
            im[f"gt8_{cc}"] = gt8_c[cc]
        in_maps.append(im)
    return in_maps


def kernel(features, labels, angles, features_orig):
    in_maps = _prep(features, labels, angles, features_orig)
    if "nc" not in _cached:
        _cached["nc"] = build_graph()
    res = run_bass_kernel_spmd(_cached["nc"], in_maps,
                               core_ids=list(range(NCORES)))
    parts = np.stack([np.asarray(r["out"]).reshape(16)
                      for r in res.results]).sum(0)
    loss = parts[0] / max(parts[1], 1.0) + 0.1 * (1.0 - parts[2] / B)
    return np.float32(loss)


if __name__ == "__main__":
    pass
